# revision 11
# baseline (speedup 1.0000x reference)
"""MBConv block with MoE routing (depthwise + pointwise expert kernels) on 8 trn2 cores.

Sharding: pure data parallel — batch 64 split 8 samples per core; all weights
replicated. Device kernel computes routing, expert-weight aggregation, expand
conv, per-sample depthwise conv, squeeze-excitation, pointwise projection,
BN folds and residual.

Depthwise engine split (per 128-channel chunk):
  - "pe" chunks: diagonal-matmul formulation on TensorE. In fp8 mode the taps
    are paired via MatmulPerfMode.DoubleRow (two kernel rows per pass).
  - "dve" chunks: scalar_tensor_tensor tap-accumulation chain on VectorE.
  - the 64-channel remainder chunk is row-packed: both spatial halves of its
    64 channels stacked into 128 partitions so all engines see full width.

Self-contained: hardcodes all shapes; host side only reshapes/prepacks weights.
"""

import os
import sys
import time

for _p in ("/opt/trn_rl_repo", os.path.expanduser("~/.axon_site/_ro/trn_rl_repo")):
    if os.path.isdir(_p) and _p not in sys.path:
        sys.path.insert(0, _p)

import contextlib

import numpy as np

import concourse.bacc as bacc
import concourse.bass as bass
import concourse.tile as tile
from concourse import mybir

F32 = mybir.dt.float32
BF16 = mybir.dt.bfloat16
FP8 = mybir.dt.float8e4
AF = mybir.ActivationFunctionType
ALU = mybir.AluOpType
AX = mybir.AxisListType
DR = mybir.MatmulPerfMode.DoubleRow

# dims (must match the problem spec)
B, CIN, H, W = 64, 96, 28, 28
NCORES = 8
BL = B // NCORES          # 8 samples per core
E = 4
HID = 576
KK = 5
T = KK * KK               # 25 taps
RED = 24                  # SE reduced dim
RHID = 24                 # routing hidden
COUT = 96
EPS = 1e-3
HW = H * W                # 784
GP = 128
PW = 32                   # padded row stride
NH = 2                    # output row halves (14 rows each)
RH = H // NH              # 14
NF = RH * W               # 392 free elems per half
ND = PW * (RH - 1) + W    # 444 padded free elems per half (diag matmul N)
NDF = PW * (H - 1) + W    # 892 padded free elems full image
XPL = 33 * PW             # 1056 padded tile length (full image)
XPL4 = 19 * PW            # 608 padded tile length (g4 packed)

# fp8 DoubleRow tap pairing: slots 0..9 are (kh, kw)+(kh+1, kw) pairs for
# kh in {0, 2}; slots 10..14 are kh=4 singles (j=1 weight is zero).
NSL = 15


def _slot_tap(s):
    if s < 10:
        kh = 2 * (s // 5)
        kw = s % 5
    else:
        kh = 4
        kw = s - 10
    return kh, kw


# chunk modes per build flavor
CFG = {
    "fp8": dict(pech=[0, 1, 2], dvech=[3], g4pe=True, fp8=True),
    "bf16": dict(pech=[0, 1], dvech=[2, 3], g4pe=True, fp8=False),
}
MODE = os.environ.get("KMODE", "fp8")


def _build_program(reps=1, ablate=(), mode=None):
    cfg = CFG[mode or MODE]
    PECH, DVECH, USE8 = cfg["pech"], cfg["dvech"], cfg["fp8"]
    XDT = FP8 if USE8 else BF16         # dtype of padded x1 tiles for PE chunks
    NPE = len(PECH) + 1                 # diag chunk count incl. packed g4
    NSLOT = NSL if USE8 else T          # lhs slots per chunk

    nc = bacc.Bacc(None, target_bir_lowering=False)

    dt = lambda name, shape: nc.dram_tensor(name, shape, F32, kind="ExternalInput")
    x_d = dt("x", [CIN, BL, HW])
    xbf_d = nc.dram_tensor("xbf", [CIN, BL, HW], BF16, kind="ExternalInput")
    expbf_d = nc.dram_tensor("expbf", [CIN, HID], BF16, kind="ExternalInput")
    identbf_d = nc.dram_tensor("identbf", [GP, GP], BF16, kind="ExternalInput")
    a1_d = dt("a1", [GP, 5])
    b1_d = dt("b1", [GP, 5])
    a2_d = dt("a2", [GP, 5])
    b2_d = dt("b2", [GP, 5])
    a3_d = dt("a3", [COUT, 1])
    b3_d = dt("b3", [COUT, 1])
    # dwTp: PE-chunk expert taps [GP, E, NPE, NSLOT*(2 if fp8)]
    dwp_cols = NSLOT * (2 if USE8 else 1)
    dwTp_d = nc.dram_tensor("dwTp", [GP, E, NPE, dwp_cols],
                            BF16, kind="ExternalInput")
    dwTv_d = dt("dwTv", [GP, E, len(DVECH), T])
    pwT_d = nc.dram_tensor("pwT", [GP, E, 5, COUT], BF16, kind="ExternalInput")
    sw1_d = dt("sw1", [GP, 5, RED])
    sw2b_d = dt("sw2b", [RED, 5, GP])
    b2se_d = dt("b2se", [GP, 5])
    rw1_d = dt("rw1", [CIN, RHID])
    rb1_d = dt("rb1", [RHID, 1])
    rw2_d = dt("rw2", [RHID, E])
    rb2_d = dt("rb2", [BL, E])
    sb1_d = dt("sb1", [RED, 1])
    y_d = nc.dram_tensor("y", [BL, COUT, HW], F32, kind="ExternalOutput")

    with tile.TileContext(nc) as tc:
        with (
            tc.tile_pool(name="consts", bufs=1) as cp,
            tc.tile_pool(name="dram", bufs=1, space="DRAM") as dp,
            tc.tile_pool(name="xpad", bufs=1) as xpp,
            tc.tile_pool(name="out2", bufs=1) as o2p,
            tc.tile_pool(name="diag", bufs=1) as dgp,
            tc.tile_pool(name="cacc", bufs=2) as accp,
            tc.tile_pool(name="wscp", bufs=2) as wsp,
            tc.tile_pool(name="outb", bufs=2) as obp,
            tc.tile_pool(name="small", bufs=2) as smp,
            tc.tile_pool(name="ppex", bufs=2, space="PSUM") as ppex,
            tc.tile_pool(name="pse", bufs=1, space="PSUM") as psep,
            tc.tile_pool(name="pdw", bufs=3, space="PSUM") as pdwp,
            tc.tile_pool(name="ppw", bufs=1, space="PSUM") as ppwp,
        ):
            # ---- persistent consts ----
            x_sb = cp.tile([CIN, BL, HW], F32, tag="x_sb")
            x_bf = cp.tile([CIN, BL, HW], BF16, tag="x_bf")
            expT = cp.tile([CIN, HID], BF16, tag="expT")
            a1 = cp.tile([GP, 5], F32, tag="a1")
            b1 = cp.tile([GP, 5], F32, tag="b1")
            a2 = cp.tile([GP, 5], F32, tag="a2")
            b2 = cp.tile([GP, 5], F32, tag="b2")
            a3 = cp.tile([COUT, 1], F32, tag="a3")
            b3 = cp.tile([COUT, 1], F32, tag="b3")
            dwTp = cp.tile([GP, E, NPE, dwp_cols], BF16, tag="dwTp")
            dwTv = cp.tile([GP, E, len(DVECH), T], F32, tag="dwTv")
            pwT = cp.tile([GP, E, 5, COUT], BF16, tag="pwT")
            sw1 = cp.tile([GP, 5, RED], F32, tag="sw1")
            sw2b = cp.tile([RED, 5, GP], F32, tag="sw2b")
            b2se = cp.tile([GP, 5], F32, tag="b2se")
            rw1 = cp.tile([CIN, RHID], F32, tag="rw1")
            rb1 = cp.tile([RHID, 1], F32, tag="rb1")
            rw2 = cp.tile([RHID, E], F32, tag="rw2")
            rb2 = cp.tile([BL, E], F32, tag="rb2")
            sb1 = cp.tile([RED, 1], F32, tag="sb1")
            ident = cp.tile([GP, GP], BF16, tag="ident")
            kernp = cp.tile([GP, NPE, BL, dwp_cols], BF16, tag="kernp")
            kernv = cp.tile([GP, len(DVECH), BL, T], F32, tag="kernv")
            rw_bc = cp.tile([GP, BL * E], F32, tag="rw_bc")

            # padded x1 tiles: 2 slots; zeros in pad regions persist
            xp_t = [
                {g: xpp.tile([GP, XPL], XDT if g in PECH else BF16,
                             tag=f"xp{s}g{g}", name=f"xp{s}g{g}")
                 for g in PECH + DVECH}
                for s in range(2)
            ]
            xp4_t = [xpp.tile([GP, XPL4], XDT, tag=f"xp4{s}", name=f"xp4{s}")
                     for s in range(2)]
            for s in range(2):
                for g in PECH + DVECH:
                    nc.gpsimd.memset(xp_t[s][g][:], 0.0)
                nc.gpsimd.memset(xp4_t[s][:], 0.0)

            # dg tiles: 2 pipeline sets x NPE chunks
            if USE8:
                dg_t = [
                    [dgp.tile([GP, NSL, 2, GP], FP8, tag=f"dg{s}c{c}",
                              name=f"dg{s}c{c}") for c in range(NPE)]
                    for s in range(2)
                ]
            else:
                dg_t = [
                    [dgp.tile([GP, T, GP], BF16, tag=f"dg{s}c{c}",
                              name=f"dg{s}c{c}") for c in range(NPE)]
                    for s in range(2)
                ]

            out2_t = [o2p.tile([GP, len(PECH) + len(DVECH), HW], BF16,
                               tag=f"o2{s}", name=f"o2{s}") for s in range(2)]
            o24_t = [o2p.tile([GP, NF], BF16, tag=f"o24{s}", name=f"o24{s}")
                     for s in range(2)]
            s_parts = [cp.tile([GP, 8], F32, tag=f"sp{b}", name=f"sp{b}")
                       for b in range(BL)]
            ident_ap = ident[:]

            def build_diags(b, slot):
                """DVE: diag lhs tiles for sample b's PE chunks."""
                npool = int(os.environ.get("KPOOLB", "2"))
                for ci in range(NPE):
                    eng = nc.gpsimd if ci >= NPE - npool else nc.vector
                    dg = dg_t[slot][ci]
                    k_ap = kernp[:, ci, b, :]
                    if USE8:
                        i_b = bass.AP(tensor=ident_ap.tensor,
                                      offset=ident_ap.offset,
                                      ap=[ident_ap.ap[0], [0, NSL], [0, 2],
                                          [1, GP]])
                        k_b = bass.AP(tensor=k_ap.tensor, offset=k_ap.offset,
                                      ap=[k_ap.ap[0], [2, NSL], [1, 2],
                                          [0, GP]])
                    else:
                        i_b = bass.AP(tensor=ident_ap.tensor,
                                      offset=ident_ap.offset,
                                      ap=[ident_ap.ap[0], [0, T], [1, GP]])
                        k_b = bass.AP(tensor=k_ap.tensor, offset=k_ap.offset,
                                      ap=[k_ap.ap[0], k_ap.ap[-1], [0, GP]])
                    eng.tensor_mul(dg[:], i_b, k_b)

            def diag_mms(xp_ap, dg, pdw_list, bases):
                """Accumulated diag matmuls; pdw_list/bases give the
                (psum tile, rhs base offset) per concurrent output group."""
                if USE8:
                    for s in range(NSL):
                        kh, kw = _slot_tap(s)
                        off = PW * kh + kw
                        for pdw, base in zip(pdw_list, bases):
                            rhs = bass.AP(tensor=xp_ap.tensor,
                                          offset=xp_ap.offset + base + off,
                                          ap=[xp_ap.ap[0], [PW, 2], [1, ND]])
                            nc.tensor.matmul(pdw[:, :ND], dg[:, s], rhs,
                                             start=(s == 0), stop=(s == NSL - 1),
                                             perf_mode=DR)
                else:
                    for t in range(T):
                        kh, kw = divmod(t, KK)
                        off = PW * kh + kw
                        for pdw, base in zip(pdw_list, bases):
                            rhs = bass.AP(tensor=xp_ap.tensor,
                                          offset=xp_ap.offset + base + off,
                                          ap=[xp_ap.ap[0], [1, ND]])
                            nc.tensor.matmul(pdw[:, :ND], dg[:, t], rhs,
                                             start=(t == 0), stop=(t == T - 1))

            def strided(ap, rows, base=0):
                return bass.AP(tensor=ap.tensor, offset=ap.offset + base,
                               ap=[ap.ap[0], [PW, rows], [1, W]])

            def emit_expand(b, slot):
                """PE: expand matmuls; ACT: bn1+silu into padded tiles."""
                for g in PECH + DVECH:
                    for h in range(NH):
                        pex = ppex.tile([GP, 512], F32, tag="pex", name="pex")
                        nc.tensor.matmul(
                            pex[:, :NF], expT[:, g * GP: (g + 1) * GP],
                            x_bf[:, b, h * NF: (h + 1) * NF],
                            start=True, stop=True)
                        if "noxact" in ablate:
                            continue
                        xpo = strided(xp_t[slot][g][:], RH, 448 * h + 2 * PW + 2)
                        nc.scalar.activation(
                            xpo, pex[:, :NF].rearrange("p (r w) -> p r w", w=W),
                            AF.Silu, bias=b1[:, g: g + 1], scale=a1[:, g: g + 1])
                # g4 packed: partitions 0:64 rows 0..15, 64:128 rows 12..27
                pex4 = ppex.tile([GP, 512], F32, tag="pex", name="pex4")
                lhs4 = expT[:, 4 * GP: 4 * GP + 64]
                nc.tensor.matmul(pex4[0:64, :448], lhs4, x_bf[:, b, 0:448],
                                 start=True, stop=True)
                nc.tensor.matmul(pex4[64:128, :448], lhs4, x_bf[:, b, 336:784],
                                 start=True, stop=True)
                for grp, dst_off in ((0, 2 * PW + 2), (1, 2)):
                    if "noxact" in ablate:
                        break
                    sl = slice(64 * grp, 64 * grp + 64)
                    xv = xp4_t[slot][sl, :]
                    dst = bass.AP(tensor=xv.tensor, offset=xv.offset + dst_off,
                                  ap=[xv.ap[0], [PW, 16], [1, W]])
                    nc.scalar.activation(
                        dst, pex4[sl, :448].rearrange("p (r w) -> p r w", w=W),
                        AF.Silu, bias=b1[sl, 4:5], scale=a1[sl, 4:5])

            def emit_dw_pe(b, slot):
                """PE diag matmuls + ACT bn2/silu into out2."""
                taps_off = "noconv" in ablate
                for gi, g in enumerate(PECH):
                    if taps_off:
                        continue
                    pdw0 = pdwp.tile([GP, 512], F32, tag="pdw", name="pdw")
                    pdw1 = pdwp.tile([GP, 512], F32, tag="pdw", name="pdwb")
                    diag_mms(xp_t[slot][g][:], dg_t[slot][gi], [pdw0, pdw1],
                             [0, 448])
                    for h, pdw in enumerate((pdw0, pdw1)):
                        if "noo2" in ablate:
                            continue
                        nc.scalar.activation(
                            out2_t[slot][:, gi, h * NF: (h + 1) * NF]
                            .rearrange("p (r w) -> p r w", w=W),
                            strided(pdw[:], RH), AF.Silu,
                            bias=b2[:, g: g + 1], scale=a2[:, g: g + 1],
                            accum_out=s_parts[b][:, 2 * gi + h: 2 * gi + h + 1])
                # g4 packed chunk: one 128-partition group
                if not taps_off:
                    pdw4 = pdwp.tile([GP, 512], F32, tag="pdw", name="pdw4")
                    diag_mms(xp4_t[slot][:], dg_t[slot][NPE - 1], [pdw4], [0])
                    if "noo2" in ablate:
                        return
                    nc.scalar.activation(
                        o24_t[slot][:].rearrange("p (r w) -> p r w", w=W),
                        strided(pdw4[:], RH), AF.Silu,
                        bias=b2[:, 4:5], scale=a2[:, 4:5],
                        accum_out=s_parts[b][:, 7:8])

            def emit_dw_dve(b, slot):
                """DVE tap chain for DVECH chunks + ACT bn2/silu."""
                for vi, g in enumerate(DVECH):
                    if "nog3" in ablate:
                        continue
                    acc = accp.tile([GP, NDF + 4], BF16, tag=f"acc{vi}",
                                    name=f"acc{vi}")
                    xpg = xp_t[slot][g][:]
                    for t in range(T):
                        kh, kw = divmod(t, KK)
                        xin = bass.AP(tensor=xpg.tensor,
                                      offset=xpg.offset + PW * kh + kw,
                                      ap=[xpg.ap[0], [1, NDF]])
                        k_ap = kernv[:, vi, b, t: t + 1]
                        if t == 0:
                            nc.vector.tensor_scalar_mul(acc[:, :NDF], xin, k_ap)
                        else:
                            nc.vector.scalar_tensor_tensor(
                                acc[:, :NDF], xin, k_ap, acc[:, :NDF],
                                op0=ALU.mult, op1=ALU.add)
                    if "noo2" in ablate:
                        continue
                    gi = len(PECH) + vi
                    nc.scalar.activation(
                        out2_t[slot][:, gi, :].rearrange("p (r w) -> p r w", w=W),
                        strided(acc[:], H), AF.Silu,
                        bias=b2[:, g: g + 1], scale=a2[:, g: g + 1],
                        accum_out=s_parts[b][:, 6 + vi - 1: 7 + vi - 1]
                        if len(DVECH) > 1 else s_parts[b][:, 6:7])

            # chunk order in out2/s_parts: PECH... then DVECH...; g4 separate.
            # s_parts slots: 2*i+h for PECH pairs, then one per DVECH, then 7=g4.
            def emit_se_pw(b, slot):
                """SE chain + pointwise matmuls + bn3/residual + store."""
                npec = len(PECH)
                s_sum = smp.tile([GP, 5], F32, tag="s_sum", name="s_sum")
                spp = s_parts[b][:, 0: 2 * npec].rearrange(
                    "p (g n) -> p g n", n=2)
                nc.vector.tensor_reduce(s_sum[:, 0:npec], spp, axis=AX.X,
                                        op=ALU.add)
                nc.vector.tensor_copy(s_sum[:, npec:5],
                                      s_parts[b][:, 6 - (len(DVECH) - 1): 8])
                # map chunk order (PECH..., DVECH..., g4) to weight col order
                colmap = PECH + DVECH + [4]
                pz = psep.tile([RED, 1], F32, tag="se", name="pz")
                for i, g in enumerate(colmap):
                    nc.tensor.matmul(pz[:], sw1[:, g], s_sum[:, i: i + 1],
                                     start=(i == 0), stop=(i == 4))
                zt = smp.tile([RED, 1], F32, tag="zt", name="zt")
                nc.scalar.activation(zt[:], pz[:], AF.Silu, bias=sb1[:], scale=1.0)
                psc = psep.tile([GP, 5], F32, tag="se", name="psc")
                for g in range(5):
                    nc.tensor.matmul(psc[:, g: g + 1], sw2b[:, g], zt[:],
                                     start=True, stop=True)
                # sigmoid(p + b) = 0.5 + 0.5*tanh(0.5*p + 0.5*b); b2se pre-halved
                sc = smp.tile([GP, 5], F32, tag="sc", name="sc")
                for g in range(5):
                    nc.scalar.activation(sc[:, g: g + 1], psc[:, g: g + 1],
                                         AF.Tanh, bias=b2se[:, g: g + 1],
                                         scale=0.5)
                nc.vector.tensor_scalar(sc[:], sc[:], 0.5, 0.5,
                                        op0=ALU.mult, op1=ALU.add)
                # wsc[:, g] = (sum_e rw[b,e] pwT[:, e, g]) * sc[:, g]
                wsc = wsp.tile([GP, 5, COUT], BF16, tag="wsc", name="wsc")
                wscb = wsp.tile([GP, 5, COUT], BF16, tag="wscb", name="wscb")
                for e in range(E):
                    s_ap = rw_bc[:, E * b + e: E * b + e + 1]
                    if e == 0:
                        nc.vector.tensor_scalar_mul(wsc[:], pwT[:, e], s_ap)
                    else:
                        nc.vector.scalar_tensor_tensor(
                            wsc[:], pwT[:, e], s_ap, wsc[:],
                            op0=ALU.mult, op1=ALU.add)
                for g in range(5):
                    nc.vector.tensor_scalar_mul(wscb[:, g], wsc[:, g],
                                                sc[:, g: g + 1])
                # pointwise projection: accumulate over chunks per half
                po = ppwp.tile([COUT, NH, 512], F32, tag="po", name="po")
                if "nopw" not in ablate:
                    for h in range(NH):
                        for i, g in enumerate(colmap):
                            if g == 4:
                                sl = slice(64 * h, 64 * h + 64)
                                nc.tensor.matmul(
                                    po[:, h, :NF], wscb[sl, g],
                                    o24_t[slot][sl, :],
                                    start=False, stop=True)
                            else:
                                nc.tensor.matmul(
                                    po[:, h, :NF], wscb[:, g],
                                    out2_t[slot][:, i, h * NF: (h + 1) * NF],
                                    start=(i == 0), stop=False)
                ob = obp.tile([COUT, HW], F32, tag="ob", name="ob")
                obt = obp.tile([COUT, HW], F32, tag="obt", name="obt")
                for h in range(NH):
                    sl = slice(h * NF, (h + 1) * NF)
                    if "nopw" in ablate:
                        nc.vector.tensor_copy(ob[:, sl], x_sb[:, b, sl])
                        continue
                    nc.scalar.activation(obt[:, sl], po[:, h, :NF], AF.Identity,
                                         bias=b3[:], scale=a3[:])
                    nc.vector.tensor_add(ob[:, sl], obt[:, sl], x_sb[:, b, sl])
                nc.sync.dma_start(y_d[b], ob[:])

            def emit_body():
                nc.sync.dma_start(x_sb[:], x_d[:])
                nc.sync.dma_start(x_bf[:], xbf_d[:])
                for t_sb, t_d in [
                    (expT, expbf_d), (a1, a1_d), (b1, b1_d), (a2, a2_d),
                    (b2, b2_d), (a3, a3_d), (b3, b3_d), (dwTp, dwTp_d),
                    (dwTv, dwTv_d), (pwT, pwT_d), (sw1, sw1_d),
                    (sw2b, sw2b_d), (b2se, b2se_d), (rw1, rw1_d),
                    (rb1, rb1_d), (rw2, rw2_d), (rb2, rb2_d), (sb1, sb1_d),
                    (ident, identbf_d),
                ]:
                    nc.sync.dma_start(t_sb[:], t_d[:])

                # routing: pool -> MLP -> softmax (samples on partitions)
                xsum = cp.tile([CIN, BL], F32, tag="xsum", name="xsum")
                nc.vector.tensor_reduce(xsum[:], x_sb[:], axis=AX.X, op=ALU.add)
                ph1 = psep.tile([RHID, BL], F32, tag="se", name="ph1")
                nc.tensor.matmul(ph1[:], rw1[:], xsum[:], start=True, stop=True)
                hdn = cp.tile([RHID, BL], F32, tag="hdn", name="hdn")
                nc.scalar.activation(hdn[:], ph1[:], AF.Relu, bias=rb1[:], scale=1.0)
                pl2 = psep.tile([BL, E], F32, tag="se", name="pl2")
                nc.tensor.matmul(pl2[:], hdn[:], rw2[:], start=True, stop=True)
                lt = cp.tile([BL, E], F32, tag="lt", name="lt")
                nc.vector.tensor_add(lt[:], pl2[:], rb2[:])
                mx = cp.tile([BL, 1], F32, tag="mx", name="mx")
                nc.vector.reduce_max(mx[:], lt[:], axis=AX.X)
                nc.vector.tensor_scalar_sub(lt[:], lt[:], mx[:])
                el = cp.tile([BL, E], F32, tag="el", name="el")
                nc.scalar.activation(el[:], lt[:], AF.Exp)
                es = cp.tile([BL, 1], F32, tag="es", name="es")
                nc.vector.reduce_sum(es[:], el[:], axis=AX.X)
                einv = cp.tile([BL, 1], F32, tag="einv", name="einv")
                nc.vector.reciprocal(einv[:], es[:])
                rwT = cp.tile([BL, E], F32, tag="rwT", name="rwT")
                nc.vector.tensor_scalar_mul(rwT[:], el[:], einv[:])
                # broadcast rw to all 128 partitions via DRAM bounce
                rw_dram = dp.tile([BL, E], F32, tag="rwd", name="rwd")
                nc.sync.dma_start(rw_dram[:], rwT[:])
                rwd_ap = rw_dram[:]
                bcast_src = bass.AP(
                    tensor=rwd_ap.tensor, offset=rwd_ap.offset,
                    ap=[[0, GP], [1, BL * E]],
                )
                nc.sync.dma_start(rw_bc[:], bcast_src)

                # expert-weight aggregation: kern for PE chunks (gpsimd) and
                # DVE chunks (vector)
                for b in range(BL):
                    kv = kernp[:, :, b, :]
                    kv3 = kernv[:, :, b, :]
                    for e in range(E):
                        s_ap = rw_bc[:, E * b + e: E * b + e + 1]
                        if e == 0:
                            nc.vector.tensor_scalar_mul(kv, dwTp[:, e], s_ap)
                            nc.vector.tensor_scalar_mul(kv3, dwTv[:, e], s_ap)
                        else:
                            nc.vector.scalar_tensor_tensor(
                                kv, dwTp[:, e], s_ap, kv, op0=ALU.mult,
                                op1=ALU.add)
                            nc.vector.scalar_tensor_tensor(
                                kv3, dwTv[:, e], s_ap, kv3, op0=ALU.mult,
                                op1=ALU.add)

                if "nodiag" not in ablate:
                    build_diags(0, 0)
                for b in range(BL):
                    slot = b % 2
                    emit_expand(b, slot)
                    if b + 1 < BL and "nodiag" not in ablate:
                        build_diags(b + 1, 1 - slot)
                    emit_dw_pe(b, slot)
                    emit_dw_dve(b, slot)
                    if b > 0:
                        emit_se_pw(b - 1, 1 - slot)
                emit_se_pw(BL - 1, (BL - 1) % 2)

            loop_ctx = (tc.For_i(0, reps, 1, hint_engines=(mybir.EngineType.PE,))
                        if reps > 1 else contextlib.nullcontext())
            with loop_ctx:
                emit_body()

    nc.compile()
    return nc


_NC = None


def _get_nc():
    global _NC
    if _NC is None:
        _NC = _build_program()
    return _NC


def _prep_maps(x, r_w1, r_b1, r_w2, r_b2, exp_w,
               bn1_g, bn1_b, bn1_m, bn1_v, dw_w,
               bn2_g, bn2_b, bn2_m, bn2_v,
               se_w1, se_b1, se_w2, se_b2, pw_w,
               bn3_g, bn3_b, bn3_m, bn3_v, mode=None):
    cfg = CFG[mode or MODE]
    PECH, DVECH, USE8 = cfg["pech"], cfg["dvech"], cfg["fp8"]
    NPE = len(PECH) + 1
    f = np.float32
    bfdt = mybir.dt.np(BF16)
    x = np.asarray(x, f).reshape(B, CIN, HW)

    def fold_bn(g, bvec, m, v):
        a = np.asarray(g, f) / np.sqrt(np.asarray(v, f) + EPS)
        return a, np.asarray(bvec, f) - np.asarray(m, f) * a

    a1v, b1v = fold_bn(bn1_g, bn1_b, bn1_m, bn1_v)
    a2v, b2v = fold_bn(bn2_g, bn2_b, bn2_m, bn2_v)
    a3v, b3v = fold_bn(bn3_g, bn3_b, bn3_m, bn3_v)

    # channel index per (partition, weight-col): cols 0..3 = chunks 0..3,
    # col 4 = packed g4 (channel duplicated across both partition halves)
    def chunk5(v):
        vv = np.asarray(v, f)
        out = np.zeros((GP, 5), f)
        for g in range(4):
            out[:, g] = vv[g * GP: (g + 1) * GP]
        out[:, 4] = vv[512 + (np.arange(GP) % 64)]
        return out

    dwf = np.asarray(dw_w, f).reshape(E, HID, T)
    dwp_cols = NSL * 2 if USE8 else T
    dwTp = np.zeros((GP, E, NPE, dwp_cols), f)
    dwTv = np.zeros((GP, E, len(DVECH), T), f)
    pcid = np.arange(GP)

    def chans(g):
        return (512 + (pcid % 64)) if g == 4 else (g * GP + pcid)

    for ci, g in enumerate(PECH + [4]):
        ch = chans(g)
        if USE8:
            for s in range(NSL):
                kh, kw = _slot_tap(s)
                dwTp[:, :, ci, 2 * s] = dwf[:, ch, kh * KK + kw].T
                if s < 10:
                    dwTp[:, :, ci, 2 * s + 1] = dwf[:, ch, (kh + 1) * KK + kw].T
        else:
            dwTp[:, :, ci, :] = dwf[:, ch, :].transpose(1, 0, 2)
    for vi, g in enumerate(DVECH):
        dwTv[:, :, vi, :] = dwf[:, chans(g), :].transpose(1, 0, 2)

    pwTa = np.zeros((GP, E, 5, COUT), f)
    sw1 = np.zeros((GP, 5, RED), f)
    sw2b = np.zeros((RED, 5, GP), f)
    b2se = np.zeros((GP, 5), f)
    for g in range(5):
        ch = chans(g)
        pwTa[:, :, g, :] = np.asarray(pw_w, f)[:, :, ch].transpose(2, 0, 1)
        sw1[:, g, :] = (np.asarray(se_w1, f)[:, ch] / HW).T
        sw2b[:, g, :] = np.asarray(se_w2, f)[ch, :].T
        b2se[:, g] = np.asarray(se_b2, f)[ch] / 2

    common = dict(
        expbf=np.ascontiguousarray(np.asarray(exp_w, f).T).astype(bfdt),
        a1=chunk5(a1v), b1=chunk5(b1v), a2=chunk5(a2v), b2=chunk5(b2v),
        a3=a3v.reshape(COUT, 1), b3=b3v.reshape(COUT, 1),
        dwTp=dwTp.astype(bfdt), dwTv=dwTv, pwT=pwTa.astype(bfdt), sw1=sw1, sw2b=sw2b,
        b2se=b2se,
        rw1=(np.asarray(r_w1, f).T / HW).copy(),
        rb1=np.asarray(r_b1, f).reshape(RHID, 1),
        rw2=np.asarray(r_w2, f).T.copy(),
        rb2=np.tile(np.asarray(r_b2, f), (BL, 1)),
        sb1=np.asarray(se_b1, f).reshape(RED, 1),
        identbf=np.eye(GP, dtype=bfdt),
    )
    out = []
    for c in range(NCORES):
        xs = np.ascontiguousarray(x[c * BL: (c + 1) * BL].transpose(1, 0, 2))
        out.append(dict(common, x=xs, xbf=xs.astype(bfdt)))
    return out


def kernel(**inputs):
    from concourse.bass_utils import run_bass_kernel_spmd

    nc = _get_nc()
    in_maps = _prep_maps(**inputs)
    res = run_bass_kernel_spmd(nc, in_maps, core_ids=list(range(NCORES)))
    y = np.concatenate([res.results[c]["y"] for c in range(NCORES)], axis=0)
    return y.reshape(B, COUT, H, W).astype(np.float32)


if __name__ == "__main__":
    t0 = time.time()
    nc = _get_nc()
    print(f"build+compile: {time.time()-t0:.1f}s")
